# revision 7
# baseline (speedup 1.0000x reference)
"""DigitCapsules dynamic-routing kernel for 8 TRN2 NeuronCores.

Strategy: shard the input-capsule axis I=4096 across 8 cores (512 each).
Per core, u_hat[b,i,c,D] is never stored globally: each routing round
re-forms u_hat pair-by-pair on the TensorEngine (block-diagonal K=32
matmuls, x stationary, W streaming, 4-way row-tiling), consumes it from
PSUM with the vector engines (agreement + softmax + c-weighting), and
reduces over i with a constant selector matmul accumulated in PSUM.
Only the tiny per-round s_j partial [64,512] is all-reduced across cores.

B=64, I=4096, C=32, D=16, d=16, 3 routing iterations.
"""

import numpy as np

import concourse.bass as bass
import concourse.mybir as mybir
from concourse import tile
from concourse.bass_utils import run_bass_kernel_spmd

B = 64
I_FULL = 4096
C = 32
D = 16
DSMALL = 16
CD = C * D  # 512
NCORES = 8
I_LOC = I_FULL // NCORES  # 512
NPAIR = I_LOC // 2  # 256
NGRP = NPAIR // 4  # 64 groups of 4 pairs (one pair per 32-row strip)
ROUNDS = 3
EPS = 1e-9

F32 = mybir.dt.float32
BF16 = mybir.dt.bfloat16


def _split_waits(nc, max_waits=1):
    """walrus in this toolchain rejects instructions carrying more than
    ~2 semaphore waits; move extras onto preceding same-engine NOPs."""
    for bb_wrap in nc.bb_map.values():
        bb = bb_wrap.bb
        newlist = []
        changed = False
        for inst in bb.instructions:
            si = inst.sync_info
            waits = list(si.on_wait) if si and si.on_wait else []
            if len(waits) > max_waits:
                extra, keep = waits[:-max_waits], waits[-max_waits:]
                k = 0
                while extra:
                    chunk, extra = extra[:max_waits], extra[max_waits:]
                    nop = mybir.InstNoOp(
                        name=f"{inst.name}-waitsplit{k}",
                        engine=inst.engine,
                        sync_info=mybir.SyncInfo(on_wait=chunk, on_update=[]),
                    )
                    nc.register_instruction(nop, overwrite=True)
                    newlist.append(nop)
                    k += 1
                inst.sync_info = mybir.SyncInfo(
                    on_wait=keep,
                    on_update=list(si.on_update) if si.on_update else [],
                )
                changed = True
            newlist.append(inst)
        if changed:
            bb.instructions = newlist


def build_bass():
    nc = bass.Bass(
        "TRN2", target_bir_lowering=False, debug=False, num_devices=NCORES
    )
    xd_ext = nc.dram_tensor("xd", [128, NGRP * 128], BF16, kind="ExternalInput").ap()
    wt_ext = nc.dram_tensor("wt", [128, NGRP * CD], BF16, kind="ExternalInput").ap()
    sel_ext = nc.dram_tensor("sel", [128, B], F32, kind="ExternalInput").ap()
    out_ext = nc.dram_tensor("out", [B, CD], F32, kind="ExternalOutput").ap()

    with tile.TileContext(nc) as tc:
        with (
            tc.tile_pool(name="persist", bufs=1) as pp,
            tc.tile_pool(name="work", bufs=4) as wp,
            tc.tile_pool(name="small", bufs=4) as sp,
            tc.tile_pool(name="uh", bufs=5, space="PSUM") as uhp,
            tc.tile_pool(name="sacc", bufs=2, space="PSUM") as saccp,
            tc.tile_pool(name="dram", bufs=2, space="DRAM") as dp,
        ):
            xd = pp.tile([128, NGRP * 128], BF16)
            wt = pp.tile([128, NGRP * CD], BF16)
            sel = pp.tile([128, B], F32)
            sel1 = pp.tile([128, B], F32)
            bij = pp.tile([128, NPAIR * C], F32)
            vbc = pp.tile([128, CD], F32)

            nc.sync.dma_start(xd[:], xd_ext)
            nc.sync.dma_start(wt[:], wt_ext)
            nc.sync.dma_start(sel[:], sel_ext)
            nc.vector.tensor_scalar_mul(sel1[:], sel[:], 1.0 / C)
            nc.vector.memset(bij[:], 0.0)

            for r in range(ROUNDS):
                s_ps = saccp.tile([B, CD], F32)
                for g in range(NGRP):
                    for st in range(4):
                        p = 4 * g + st
                        uh = uhp.tile([128, CD], F32)
                        nc.tensor.matmul(
                            uh[:],
                            lhsT=xd[32 * st : 32 * st + 32, g * 128 : (g + 1) * 128],
                            rhs=wt[32 * st : 32 * st + 32, g * CD : (g + 1) * CD],
                            start=True,
                            stop=True,
                            tile_position=(32 * st, 0),
                        )
                        first = p == 0
                        last = p == NPAIR - 1
                        if r == 0:
                            # uniform c_ij = 1/C; selector matmul carries 1/C
                            y = wp.tile([128, CD], F32, tag="y")
                            if st % 2 == 0:
                                nc.vector.tensor_scalar_mul(y[:], uh[:], 1.0)
                            else:
                                nc.scalar.activation(
                                    y[:], uh[:], mybir.ActivationFunctionType.Copy
                                )
                            nc.tensor.matmul(
                                s_ps[:], lhsT=sel1[:], rhs=y[:],
                                start=first, stop=last, skip_group_check=True,
                            )
                        else:
                            bsl = bij[:, p * C : (p + 1) * C]
                            # agreement: a = sum_D u_hat * v  -> b_ij += a
                            tmp = wp.tile([128, CD], F32, tag="tmp")
                            nc.vector.tensor_tensor(
                                out=tmp[:], in0=uh[:], in1=vbc[:],
                                op=mybir.AluOpType.mult,
                            )
                            a = sp.tile([128, C], F32, tag="a")
                            nc.vector.tensor_reduce(
                                out=a[:],
                                in_=tmp[:].rearrange("p (c d) -> p c d", d=D),
                                axis=mybir.AxisListType.X,
                                op=mybir.AluOpType.add,
                            )
                            nc.vector.tensor_tensor(
                                out=bsl, in0=bsl, in1=a[:], op=mybir.AluOpType.add
                            )
                            # softmax over c (no max-sub: |b_ij| stays small)
                            e = sp.tile([128, C], F32, tag="e")
                            nc.scalar.activation(
                                e[:], bsl, mybir.ActivationFunctionType.Exp
                            )
                            sm = sp.tile([128, 1], F32, tag="sm")
                            nc.vector.tensor_reduce(
                                out=sm[:], in_=e[:],
                                axis=mybir.AxisListType.X, op=mybir.AluOpType.add,
                            )
                            rs = sp.tile([128, 1], F32, tag="rs")
                            nc.vector.reciprocal(rs[:], sm[:])
                            cn = sp.tile([128, C], F32, tag="cn")
                            nc.vector.tensor_scalar_mul(cn[:], e[:], rs[:])
                            # y = c_ij * u_hat  (c broadcast along D)
                            y = wp.tile([128, CD], F32, tag="y")
                            nc.vector.tensor_tensor(
                                out=y[:].rearrange("p (c d) -> p c d", d=D),
                                in0=uh[:].rearrange("p (c d) -> p c d", d=D),
                                in1=cn[:].unsqueeze(2).broadcast_to([128, C, D]),
                                op=mybir.AluOpType.mult,
                            )
                            nc.tensor.matmul(
                                s_ps[:], lhsT=sel[:], rhs=y[:],
                                start=first, stop=last, skip_group_check=True,
                            )

                # evacuate s partial, all-reduce across the 8 cores
                s_sb = wp.tile([B, CD], F32, tag="s_sb")
                nc.scalar.activation(
                    s_sb[:], s_ps[:], mybir.ActivationFunctionType.Copy
                )
                ccin = dp.tile([B, CD], F32, tag="ccin")
                ccout = dp.tile([B, CD], F32, tag="ccout")
                nc.gpsimd.dma_start(ccin[:], s_sb[:])
                nc.gpsimd.collective_compute(
                    "AllReduce",
                    mybir.AluOpType.add,
                    replica_groups=[list(range(NCORES))],
                    ins=[ccin[:].opt()],
                    outs=[ccout[:].opt()],
                )
                s2 = wp.tile([B, CD], F32, tag="s2")
                nc.gpsimd.dma_start(s2[:], ccout[:])

                # squash: v = s / (1 + n2) / sqrt(n2 + eps), n2 = sum_D s^2
                sq = wp.tile([B, CD], F32, tag="sq")
                nc.vector.tensor_tensor(
                    out=sq[:], in0=s2[:], in1=s2[:], op=mybir.AluOpType.mult
                )
                n2 = sp.tile([B, C], F32, tag="n2")
                nc.vector.tensor_reduce(
                    out=n2[:],
                    in_=sq[:].rearrange("p (c d) -> p c d", d=D),
                    axis=mybir.AxisListType.X,
                    op=mybir.AluOpType.add,
                )
                n2e = sp.tile([B, C], F32, tag="n2e")
                nc.vector.tensor_scalar_add(n2e[:], n2[:], EPS)
                rt = sp.tile([B, C], F32, tag="rt")
                nc.scalar.activation(
                    rt[:], n2e[:], mybir.ActivationFunctionType.Sqrt
                )
                on2 = sp.tile([B, C], F32, tag="on2")
                nc.vector.tensor_scalar_add(on2[:], n2[:], 1.0)
                den = sp.tile([B, C], F32, tag="den")
                nc.vector.tensor_tensor(
                    out=den[:], in0=rt[:], in1=on2[:], op=mybir.AluOpType.mult
                )
                scl = sp.tile([B, C], F32, tag="scl")
                nc.vector.reciprocal(scl[:], den[:])
                v_sb = wp.tile([B, CD], F32, tag="v_sb")
                nc.vector.tensor_tensor(
                    out=v_sb[:].rearrange("p (c d) -> p c d", d=D),
                    in0=s2[:].rearrange("p (c d) -> p c d", d=D),
                    in1=scl[:].unsqueeze(2).broadcast_to([B, C, D]),
                    op=mybir.AluOpType.mult,
                )
                if r < ROUNDS - 1:
                    # broadcast v to both partition halves for next round
                    nc.sync.dma_start(vbc[0:B, :], v_sb[:])
                    nc.sync.dma_start(vbc[B : 2 * B, :], v_sb[:])
                else:
                    nc.sync.dma_start(out_ext, v_sb[:])
    _split_waits(nc)
    return nc


def _prep_core_inputs(x_np, w_np, core):
    """x_np [B, I, d] f32; w_np [I, C, D, d] f32 -> per-core bf16 operands."""
    import ml_dtypes

    lo = core * I_LOC
    xk = x_np[:, lo : lo + I_LOC, :]  # [B, 512, 16]
    wk = w_np[lo : lo + I_LOC]  # [512, C, D, d]

    # W pair tiles: [NPAIR, 32, CD]; rows 0:16 = i0 (d-major), 16:32 = i1
    wt = np.zeros((NPAIR, 32, CD), dtype=np.float32)
    w_dcd = wk.transpose(0, 3, 1, 2).reshape(I_LOC, DSMALL, CD)  # [i, d, (c D)]
    wt[:, 0:DSMALL, :] = w_dcd[0::2]
    wt[:, DSMALL:32, :] = w_dcd[1::2]
    # strip-pack: pair p=4g+s -> partitions [32s,32s+32), free block g
    wsb = wt.reshape(NGRP, 4, 32, CD).transpose(1, 2, 0, 3).reshape(128, NGRP * CD)

    # x block-diag pair tiles: [NPAIR, 32, 128]
    xdg = np.zeros((NPAIR, 32, 128), dtype=np.float32)
    xT = xk.transpose(1, 2, 0)  # [i, d, B]
    xdg[:, 0:DSMALL, 0:B] = xT[0::2]
    xdg[:, DSMALL:32, B : 2 * B] = xT[1::2]
    xsb = xdg.reshape(NGRP, 4, 32, 128).transpose(1, 2, 0, 3).reshape(128, NGRP * 128)

    return {
        "xd": xsb.astype(ml_dtypes.bfloat16),
        "wt": wsb.astype(ml_dtypes.bfloat16),
    }


_NC_CACHE = {}


def kernel(x: np.ndarray, weights: np.ndarray) -> np.ndarray:
    import ml_dtypes

    x = np.asarray(x, dtype=np.float32)
    w = np.asarray(weights, dtype=np.float32)[0]  # [I, C, D, d]

    if "nc" not in _NC_CACHE:
        _NC_CACHE["nc"] = build_bass()
    nc = _NC_CACHE["nc"]

    selmask = np.zeros((128, B), dtype=np.float32)
    for p in range(128):
        selmask[p, p % B] = 1.0

    in_maps = []
    for core in range(NCORES):
        m = _prep_core_inputs(x, w, core)
        m["sel"] = selmask
        in_maps.append(m)

    res = run_bass_kernel_spmd(nc, in_maps, list(range(NCORES)))
    out = np.asarray(res.results[0]["out"], dtype=np.float32)  # [B, CD]
    return out.reshape(B, C, D)


# revision 8
# speedup vs baseline: 1.1800x; 1.1800x over previous
"""DigitCapsules dynamic-routing kernel for 8 TRN2 NeuronCores.

Strategy: shard the input-capsule axis I=4096 across 8 cores (512 each).
Per core, u_hat[b,i,c,D] is never stored globally: each routing round
re-forms u_hat pair-by-pair on the TensorEngine (block-diagonal K=32
matmuls, x stationary, W streaming, 4-way row-tiling), consumes it from
PSUM with the vector engines (agreement + softmax + c-weighting), and
reduces over i with a constant selector matmul accumulated in PSUM.
Only the tiny per-round s_j partial [64,512] is all-reduced across cores.

B=64, I=4096, C=32, D=16, d=16, 3 routing iterations.
"""

import numpy as np

import concourse.bass as bass
import concourse.mybir as mybir
from concourse import tile
from concourse.bass_utils import run_bass_kernel_spmd

B = 64
I_FULL = 4096
C = 32
D = 16
DSMALL = 16
CD = C * D  # 512
NCORES = 8
I_LOC = I_FULL // NCORES  # 512
NPAIR = I_LOC // 2  # 256
NGRP = NPAIR // 4  # 64 groups of 4 pairs (one pair per 32-row strip)
ROUNDS = 3
EPS = 1e-9

F32 = mybir.dt.float32
BF16 = mybir.dt.bfloat16


def _split_waits(nc, max_waits=1):
    """walrus in this toolchain rejects instructions carrying more than
    ~2 semaphore waits; move extras onto preceding same-engine NOPs."""
    for bb_wrap in nc.bb_map.values():
        bb = bb_wrap.bb
        newlist = []
        changed = False
        for inst in bb.instructions:
            si = inst.sync_info
            waits = list(si.on_wait) if si and si.on_wait else []
            if len(waits) > max_waits:
                extra, keep = waits[:-max_waits], waits[-max_waits:]
                k = 0
                while extra:
                    chunk, extra = extra[:max_waits], extra[max_waits:]
                    nop = mybir.InstNoOp(
                        name=f"{inst.name}-waitsplit{k}",
                        engine=inst.engine,
                        sync_info=mybir.SyncInfo(on_wait=chunk, on_update=[]),
                    )
                    nc.register_instruction(nop, overwrite=True)
                    newlist.append(nop)
                    k += 1
                inst.sync_info = mybir.SyncInfo(
                    on_wait=keep,
                    on_update=list(si.on_update) if si.on_update else [],
                )
                changed = True
            newlist.append(inst)
        if changed:
            bb.instructions = newlist


def build_bass():
    nc = bass.Bass(
        "TRN2", target_bir_lowering=False, debug=False, num_devices=NCORES
    )
    xd_ext = nc.dram_tensor("xd", [128, NGRP * 128], BF16, kind="ExternalInput").ap()
    wt_ext = nc.dram_tensor("wt", [128, NGRP * CD], BF16, kind="ExternalInput").ap()
    sel_ext = nc.dram_tensor("sel", [128, B], BF16, kind="ExternalInput").ap()
    out_ext = nc.dram_tensor("out", [B, CD], F32, kind="ExternalOutput").ap()

    with tile.TileContext(nc) as tc:
        with (
            tc.tile_pool(name="persist", bufs=1) as pp,
            tc.tile_pool(name="work", bufs=4) as wp,
            tc.tile_pool(name="small", bufs=4) as sp,
            tc.tile_pool(name="uh", bufs=6, space="PSUM") as uhp,
            tc.tile_pool(name="sacc", bufs=2, space="PSUM") as saccp,
            tc.tile_pool(name="dram", bufs=2, space="DRAM") as dp,
        ):
            xd = pp.tile([128, NGRP * 128], BF16)
            wt = pp.tile([128, NGRP * CD], BF16)
            sel = pp.tile([128, B], BF16)
            sel1 = pp.tile([128, B], BF16)
            bij = pp.tile([128, NPAIR * C], F32)
            vbc = pp.tile([128, CD], F32)

            nc.sync.dma_start(xd[:], xd_ext)
            nc.sync.dma_start(wt[:], wt_ext)
            nc.sync.dma_start(sel[:], sel_ext)
            nc.vector.tensor_scalar_mul(sel1[:], sel[:], 1.0 / C)
            nc.vector.memset(bij[:], 0.0)

            for r in range(ROUNDS):
                s_ps = saccp.tile([B, CD], F32)
                for g in range(NGRP):
                    for st in range(4):
                        p = 4 * g + st
                        uh = uhp.tile([128, CD], F32)
                        nc.tensor.matmul(
                            uh[:],
                            lhsT=xd[32 * st : 32 * st + 32, g * 128 : (g + 1) * 128],
                            rhs=wt[32 * st : 32 * st + 32, g * CD : (g + 1) * CD],
                            start=True,
                            stop=True,
                            tile_position=(32 * st, 0),
                        )
                        first = p == 0
                        last = p == NPAIR - 1
                        if r == 0:
                            # uniform c_ij = 1/C; selector matmul carries 1/C
                            y = wp.tile([128, CD], BF16, tag="y")
                            if st % 2 == 0:
                                nc.vector.tensor_scalar_mul(y[:], uh[:], 1.0)
                            else:
                                nc.scalar.activation(
                                    y[:], uh[:], mybir.ActivationFunctionType.Copy
                                )
                            nc.tensor.matmul(
                                s_ps[:], lhsT=sel1[:], rhs=y[:],
                                start=first, stop=last, skip_group_check=True,
                            )
                        else:
                            bsl = bij[:, p * C : (p + 1) * C]
                            # agreement: a = sum_D u_hat * v  -> b_ij += a
                            tmp = wp.tile([128, CD], BF16, tag="tmp")
                            nc.vector.tensor_tensor(
                                out=tmp[:], in0=uh[:], in1=vbc[:],
                                op=mybir.AluOpType.mult,
                            )
                            a = sp.tile([128, C], F32, tag="a")
                            nc.vector.tensor_reduce(
                                out=a[:],
                                in_=tmp[:].rearrange("p (c d) -> p c d", d=D),
                                axis=mybir.AxisListType.X,
                                op=mybir.AluOpType.add,
                            )
                            nc.gpsimd.tensor_tensor(
                                out=bsl, in0=bsl, in1=a[:], op=mybir.AluOpType.add
                            )
                            # softmax over c (no max-sub: |b_ij| stays small)
                            e = sp.tile([128, C], F32, tag="e")
                            nc.scalar.activation(
                                e[:], bsl, mybir.ActivationFunctionType.Exp
                            )
                            sm = sp.tile([128, 1], F32, tag="sm")
                            nc.vector.tensor_reduce(
                                out=sm[:], in_=e[:],
                                axis=mybir.AxisListType.X, op=mybir.AluOpType.add,
                            )
                            rs = sp.tile([128, 1], F32, tag="rs")
                            nc.vector.reciprocal(rs[:], sm[:])
                            cn = sp.tile([128, C], F32, tag="cn")
                            nc.gpsimd.tensor_scalar_mul(cn[:], e[:], rs[:])
                            # y = c_ij * u_hat  (c broadcast along D)
                            y = wp.tile([128, CD], BF16, tag="y")
                            nc.vector.tensor_tensor(
                                out=y[:].rearrange("p (c d) -> p c d", d=D),
                                in0=uh[:].rearrange("p (c d) -> p c d", d=D),
                                in1=cn[:].unsqueeze(2).broadcast_to([128, C, D]),
                                op=mybir.AluOpType.mult,
                            )
                            nc.tensor.matmul(
                                s_ps[:], lhsT=sel[:], rhs=y[:],
                                start=first, stop=last, skip_group_check=True,
                            )

                # evacuate s partial, all-reduce across the 8 cores
                s_sb = wp.tile([B, CD], F32, tag="s_sb")
                nc.scalar.activation(
                    s_sb[:], s_ps[:], mybir.ActivationFunctionType.Copy
                )
                ccin = dp.tile([B, CD], F32, tag="ccin")
                ccout = dp.tile([B, CD], F32, tag="ccout")
                nc.gpsimd.dma_start(ccin[:], s_sb[:])
                nc.gpsimd.collective_compute(
                    "AllReduce",
                    mybir.AluOpType.add,
                    replica_groups=[list(range(NCORES))],
                    ins=[ccin[:].opt()],
                    outs=[ccout[:].opt()],
                )
                s2 = wp.tile([B, CD], F32, tag="s2")
                nc.gpsimd.dma_start(s2[:], ccout[:])

                # squash: v = s / (1 + n2) / sqrt(n2 + eps), n2 = sum_D s^2
                sq = wp.tile([B, CD], F32, tag="sq")
                nc.vector.tensor_tensor(
                    out=sq[:], in0=s2[:], in1=s2[:], op=mybir.AluOpType.mult
                )
                n2 = sp.tile([B, C], F32, tag="n2")
                nc.vector.tensor_reduce(
                    out=n2[:],
                    in_=sq[:].rearrange("p (c d) -> p c d", d=D),
                    axis=mybir.AxisListType.X,
                    op=mybir.AluOpType.add,
                )
                n2e = sp.tile([B, C], F32, tag="n2e")
                nc.vector.tensor_scalar_add(n2e[:], n2[:], EPS)
                rt = sp.tile([B, C], F32, tag="rt")
                nc.scalar.activation(
                    rt[:], n2e[:], mybir.ActivationFunctionType.Sqrt
                )
                on2 = sp.tile([B, C], F32, tag="on2")
                nc.vector.tensor_scalar_add(on2[:], n2[:], 1.0)
                den = sp.tile([B, C], F32, tag="den")
                nc.vector.tensor_tensor(
                    out=den[:], in0=rt[:], in1=on2[:], op=mybir.AluOpType.mult
                )
                scl = sp.tile([B, C], F32, tag="scl")
                nc.vector.reciprocal(scl[:], den[:])
                v_sb = wp.tile([B, CD], F32, tag="v_sb")
                nc.vector.tensor_tensor(
                    out=v_sb[:].rearrange("p (c d) -> p c d", d=D),
                    in0=s2[:].rearrange("p (c d) -> p c d", d=D),
                    in1=scl[:].unsqueeze(2).broadcast_to([B, C, D]),
                    op=mybir.AluOpType.mult,
                )
                if r < ROUNDS - 1:
                    # broadcast v to both partition halves for next round
                    nc.sync.dma_start(vbc[0:B, :], v_sb[:])
                    nc.sync.dma_start(vbc[B : 2 * B, :], v_sb[:])
                else:
                    nc.sync.dma_start(out_ext, v_sb[:])
    _split_waits(nc)
    return nc


def _prep_core_inputs(x_np, w_np, core):
    """x_np [B, I, d] f32; w_np [I, C, D, d] f32 -> per-core bf16 operands."""
    import ml_dtypes

    lo = core * I_LOC
    xk = x_np[:, lo : lo + I_LOC, :]  # [B, 512, 16]
    wk = w_np[lo : lo + I_LOC]  # [512, C, D, d]

    # W pair tiles: [NPAIR, 32, CD]; rows 0:16 = i0 (d-major), 16:32 = i1
    wt = np.zeros((NPAIR, 32, CD), dtype=np.float32)
    w_dcd = wk.transpose(0, 3, 1, 2).reshape(I_LOC, DSMALL, CD)  # [i, d, (c D)]
    wt[:, 0:DSMALL, :] = w_dcd[0::2]
    wt[:, DSMALL:32, :] = w_dcd[1::2]
    # strip-pack: pair p=4g+s -> partitions [32s,32s+32), free block g
    wsb = wt.reshape(NGRP, 4, 32, CD).transpose(1, 2, 0, 3).reshape(128, NGRP * CD)

    # x block-diag pair tiles: [NPAIR, 32, 128]
    xdg = np.zeros((NPAIR, 32, 128), dtype=np.float32)
    xT = xk.transpose(1, 2, 0)  # [i, d, B]
    xdg[:, 0:DSMALL, 0:B] = xT[0::2]
    xdg[:, DSMALL:32, B : 2 * B] = xT[1::2]
    xsb = xdg.reshape(NGRP, 4, 32, 128).transpose(1, 2, 0, 3).reshape(128, NGRP * 128)

    return {
        "xd": xsb.astype(ml_dtypes.bfloat16),
        "wt": wsb.astype(ml_dtypes.bfloat16),
    }


_NC_CACHE = {}


def kernel(x: np.ndarray, weights: np.ndarray) -> np.ndarray:
    import ml_dtypes

    x = np.asarray(x, dtype=np.float32)
    w = np.asarray(weights, dtype=np.float32)[0]  # [I, C, D, d]

    if "nc" not in _NC_CACHE:
        _NC_CACHE["nc"] = build_bass()
    nc = _NC_CACHE["nc"]

    selmask = np.zeros((128, B), dtype=np.float32)
    for p in range(128):
        selmask[p, p % B] = 1.0

    in_maps = []
    for core in range(NCORES):
        m = _prep_core_inputs(x, w, core)
        m["sel"] = selmask.astype(ml_dtypes.bfloat16)
        in_maps.append(m)

    res = run_bass_kernel_spmd(nc, in_maps, list(range(NCORES)))
    out = np.asarray(res.results[0]["out"], dtype=np.float32)  # [B, CD]
    return out.reshape(B, C, D)


# revision 9
# speedup vs baseline: 1.3475x; 1.1420x over previous
"""DigitCapsules dynamic-routing kernel for 8 TRN2 NeuronCores.

Strategy: shard the input-capsule axis I=4096 across 8 cores (512 each).
Per core, u_hat[b,i,c,D] is never stored globally: each routing round
re-forms u_hat pair-by-pair on the TensorEngine (block-diagonal K=32
matmuls, x stationary, W streaming, 4-way row-tiling), consumes it from
PSUM with the vector engines (agreement + softmax + c-weighting), and
reduces over i with a constant selector matmul accumulated in PSUM.
Only the tiny per-round s_j partial [64,512] is all-reduced across cores.

B=64, I=4096, C=32, D=16, d=16, 3 routing iterations.
"""

import numpy as np

import concourse.bass as bass
import concourse.mybir as mybir
from concourse import tile
from concourse.bass_utils import run_bass_kernel_spmd

B = 64
I_FULL = 4096
C = 32
D = 16
DSMALL = 16
CD = C * D  # 512
NCORES = 8
I_LOC = I_FULL // NCORES  # 512
NPAIR = I_LOC // 2  # 256
NGRP = NPAIR // 4  # 64 groups of 4 pairs (one pair per 32-row strip)
ROUNDS = 3
EPS = 1e-9

F32 = mybir.dt.float32
BF16 = mybir.dt.bfloat16


def _split_waits(nc, max_waits=1):
    """walrus in this toolchain rejects instructions carrying more than
    ~2 semaphore waits; move extras onto preceding same-engine NOPs."""
    for bb_wrap in nc.bb_map.values():
        bb = bb_wrap.bb
        newlist = []
        changed = False
        for inst in bb.instructions:
            si = inst.sync_info
            waits = list(si.on_wait) if si and si.on_wait else []
            if len(waits) > max_waits:
                extra, keep = waits[:-max_waits], waits[-max_waits:]
                k = 0
                while extra:
                    chunk, extra = extra[:max_waits], extra[max_waits:]
                    nop = mybir.InstNoOp(
                        name=f"{inst.name}-waitsplit{k}",
                        engine=inst.engine,
                        sync_info=mybir.SyncInfo(on_wait=chunk, on_update=[]),
                    )
                    nc.register_instruction(nop, overwrite=True)
                    newlist.append(nop)
                    k += 1
                inst.sync_info = mybir.SyncInfo(
                    on_wait=keep,
                    on_update=list(si.on_update) if si.on_update else [],
                )
                changed = True
            newlist.append(inst)
        if changed:
            bb.instructions = newlist


def build_bass():
    nc = bass.Bass(
        "TRN2", target_bir_lowering=False, debug=False, num_devices=NCORES
    )
    xd_ext = nc.dram_tensor("xd", [128, NGRP * 128], BF16, kind="ExternalInput").ap()
    wt_ext = nc.dram_tensor("wt", [128, NGRP * CD], BF16, kind="ExternalInput").ap()
    sel_ext = nc.dram_tensor("sel", [128, B], BF16, kind="ExternalInput").ap()
    out_ext = nc.dram_tensor("out", [B, CD], F32, kind="ExternalOutput").ap()

    with tile.TileContext(nc) as tc:
        with (
            tc.tile_pool(name="persist", bufs=1) as pp,
            tc.tile_pool(name="work", bufs=4) as wp,
            tc.tile_pool(name="small", bufs=4) as sp,
            tc.tile_pool(name="uh", bufs=6, space="PSUM") as uhp,
            tc.tile_pool(name="sacc", bufs=2, space="PSUM") as saccp,
            tc.tile_pool(name="dram", bufs=2, space="DRAM") as dp,
        ):
            xd = pp.tile([128, NGRP * 128], BF16)
            wt = pp.tile([128, NGRP * CD], BF16)
            sel = pp.tile([128, B], BF16)
            sel1 = pp.tile([128, B], BF16)
            bij = pp.tile([128, NPAIR * C], F32)
            vbc = pp.tile([128, CD], F32)

            nc.sync.dma_start(xd[:], xd_ext)
            nc.sync.dma_start(wt[:], wt_ext)
            nc.sync.dma_start(sel[:], sel_ext)
            nc.vector.tensor_scalar_mul(sel1[:], sel[:], 1.0 / C)
            nc.vector.memset(bij[:], 0.0)

            for r in range(ROUNDS):
                s_ps = saccp.tile([B, CD], F32)
                for g in range(NGRP):
                    for st in range(4):
                        p = 4 * g + st
                        uh = uhp.tile([128, CD], F32)
                        nc.tensor.matmul(
                            uh[:],
                            lhsT=xd[32 * st : 32 * st + 32, g * 128 : (g + 1) * 128],
                            rhs=wt[32 * st : 32 * st + 32, g * CD : (g + 1) * CD],
                            start=True,
                            stop=True,
                            tile_position=(32 * st, 0),
                        )
                        first = p == 0
                        last = p == NPAIR - 1
                        if r == 0:
                            # uniform c_ij = 1/C; selector matmul carries 1/C
                            y = wp.tile([128, CD], BF16, tag="y")
                            if st % 2 == 0:
                                nc.vector.tensor_scalar_mul(y[:], uh[:], 1.0)
                            else:
                                nc.scalar.activation(
                                    y[:], uh[:], mybir.ActivationFunctionType.Copy
                                )
                            # (evac split DVE/ACT keeps both engines busy)
                            nc.tensor.matmul(
                                s_ps[:], lhsT=sel1[:], rhs=y[:],
                                start=first, stop=last, skip_group_check=True,
                            )
                        else:
                            bsl = bij[:, p * C : (p + 1) * C]
                            # agreement: a = sum_D u_hat * v  -> b_ij += a
                            tmp = wp.tile([128, CD], BF16, tag="tmp")
                            nc.vector.tensor_tensor(
                                out=tmp[:], in0=uh[:], in1=vbc[:],
                                op=mybir.AluOpType.mult,
                            )
                            a = sp.tile([128, C], F32, tag="a")
                            nc.vector.tensor_reduce(
                                out=a[:],
                                in_=tmp[:].rearrange("p (c d) -> p c d", d=D),
                                axis=mybir.AxisListType.X,
                                op=mybir.AluOpType.add,
                            )
                            nc.gpsimd.tensor_tensor(
                                out=bsl, in0=bsl, in1=a[:], op=mybir.AluOpType.add
                            )
                            # softmax over c (no max-sub: |b_ij| stays small)
                            e = sp.tile([128, C], F32, tag="e")
                            nc.scalar.activation(
                                e[:], bsl, mybir.ActivationFunctionType.Exp
                            )
                            sm = sp.tile([128, 1], F32, tag="sm")
                            nc.vector.tensor_reduce(
                                out=sm[:], in_=e[:],
                                axis=mybir.AxisListType.X, op=mybir.AluOpType.add,
                            )
                            rs = sp.tile([128, 1], F32, tag="rs")
                            nc.vector.reciprocal(rs[:], sm[:])
                            cn = sp.tile([128, C], F32, tag="cn")
                            nc.scalar.activation(
                                cn[:], e[:], mybir.ActivationFunctionType.Copy,
                                scale=rs[:],
                            )
                            # y = c_ij * u_hat  (c broadcast along D)
                            y = wp.tile([128, CD], BF16, tag="y")
                            nc.vector.tensor_tensor(
                                out=y[:].rearrange("p (c d) -> p c d", d=D),
                                in0=uh[:].rearrange("p (c d) -> p c d", d=D),
                                in1=cn[:].unsqueeze(2).broadcast_to([128, C, D]),
                                op=mybir.AluOpType.mult,
                            )
                            nc.tensor.matmul(
                                s_ps[:], lhsT=sel[:], rhs=y[:],
                                start=first, stop=last, skip_group_check=True,
                            )

                # evacuate s partial, all-reduce across the 8 cores
                s_sb = wp.tile([B, CD], F32, tag="s_sb")
                nc.scalar.activation(
                    s_sb[:], s_ps[:], mybir.ActivationFunctionType.Copy
                )
                ccin = dp.tile([B, CD], F32, tag="ccin")
                ccout = dp.tile([B, CD], F32, tag="ccout")
                nc.gpsimd.dma_start(ccin[:], s_sb[:])
                nc.gpsimd.collective_compute(
                    "AllReduce",
                    mybir.AluOpType.add,
                    replica_groups=[list(range(NCORES))],
                    ins=[ccin[:].opt()],
                    outs=[ccout[:].opt()],
                )
                s2 = wp.tile([B, CD], F32, tag="s2")
                nc.gpsimd.dma_start(s2[:], ccout[:])

                # squash: v = s / (1 + n2) / sqrt(n2 + eps), n2 = sum_D s^2
                sq = wp.tile([B, CD], F32, tag="sq")
                nc.vector.tensor_tensor(
                    out=sq[:], in0=s2[:], in1=s2[:], op=mybir.AluOpType.mult
                )
                n2 = sp.tile([B, C], F32, tag="n2")
                nc.vector.tensor_reduce(
                    out=n2[:],
                    in_=sq[:].rearrange("p (c d) -> p c d", d=D),
                    axis=mybir.AxisListType.X,
                    op=mybir.AluOpType.add,
                )
                n2e = sp.tile([B, C], F32, tag="n2e")
                nc.vector.tensor_scalar_add(n2e[:], n2[:], EPS)
                rt = sp.tile([B, C], F32, tag="rt")
                nc.scalar.activation(
                    rt[:], n2e[:], mybir.ActivationFunctionType.Sqrt
                )
                on2 = sp.tile([B, C], F32, tag="on2")
                nc.vector.tensor_scalar_add(on2[:], n2[:], 1.0)
                den = sp.tile([B, C], F32, tag="den")
                nc.vector.tensor_tensor(
                    out=den[:], in0=rt[:], in1=on2[:], op=mybir.AluOpType.mult
                )
                scl = sp.tile([B, C], F32, tag="scl")
                nc.vector.reciprocal(scl[:], den[:])
                v_sb = wp.tile([B, CD], F32, tag="v_sb")
                nc.vector.tensor_tensor(
                    out=v_sb[:].rearrange("p (c d) -> p c d", d=D),
                    in0=s2[:].rearrange("p (c d) -> p c d", d=D),
                    in1=scl[:].unsqueeze(2).broadcast_to([B, C, D]),
                    op=mybir.AluOpType.mult,
                )
                if r < ROUNDS - 1:
                    # broadcast v to both partition halves for next round
                    nc.sync.dma_start(vbc[0:B, :], v_sb[:])
                    nc.sync.dma_start(vbc[B : 2 * B, :], v_sb[:])
                else:
                    nc.sync.dma_start(out_ext, v_sb[:])
    _split_waits(nc)
    return nc


def _prep_core_inputs(x_np, w_np, core):
    """x_np [B, I, d] f32; w_np [I, C, D, d] f32 -> per-core bf16 operands."""
    import ml_dtypes

    lo = core * I_LOC
    xk = x_np[:, lo : lo + I_LOC, :]  # [B, 512, 16]
    wk = w_np[lo : lo + I_LOC]  # [512, C, D, d]

    # W pair tiles: [NPAIR, 32, CD]; rows 0:16 = i0 (d-major), 16:32 = i1
    wt = np.zeros((NPAIR, 32, CD), dtype=np.float32)
    w_dcd = wk.transpose(0, 3, 1, 2).reshape(I_LOC, DSMALL, CD)  # [i, d, (c D)]
    wt[:, 0:DSMALL, :] = w_dcd[0::2]
    wt[:, DSMALL:32, :] = w_dcd[1::2]
    # strip-pack: pair p=4g+s -> partitions [32s,32s+32), free block g
    wsb = wt.reshape(NGRP, 4, 32, CD).transpose(1, 2, 0, 3).reshape(128, NGRP * CD)

    # x block-diag pair tiles: [NPAIR, 32, 128]
    xdg = np.zeros((NPAIR, 32, 128), dtype=np.float32)
    xT = xk.transpose(1, 2, 0)  # [i, d, B]
    xdg[:, 0:DSMALL, 0:B] = xT[0::2]
    xdg[:, DSMALL:32, B : 2 * B] = xT[1::2]
    xsb = xdg.reshape(NGRP, 4, 32, 128).transpose(1, 2, 0, 3).reshape(128, NGRP * 128)

    return {
        "xd": xsb.astype(ml_dtypes.bfloat16),
        "wt": wsb.astype(ml_dtypes.bfloat16),
    }


_NC_CACHE = {}


def kernel(x: np.ndarray, weights: np.ndarray) -> np.ndarray:
    import ml_dtypes

    x = np.asarray(x, dtype=np.float32)
    w = np.asarray(weights, dtype=np.float32)[0]  # [I, C, D, d]

    if "nc" not in _NC_CACHE:
        _NC_CACHE["nc"] = build_bass()
    nc = _NC_CACHE["nc"]

    selmask = np.zeros((128, B), dtype=np.float32)
    for p in range(128):
        selmask[p, p % B] = 1.0

    in_maps = []
    for core in range(NCORES):
        m = _prep_core_inputs(x, w, core)
        m["sel"] = selmask.astype(ml_dtypes.bfloat16)
        in_maps.append(m)

    res = run_bass_kernel_spmd(nc, in_maps, list(range(NCORES)))
    out = np.asarray(res.results[0]["out"], dtype=np.float32)  # [B, CD]
    return out.reshape(B, C, D)
